# revision 1
# baseline (speedup 1.0000x reference)
"""BoxFilter (9x9 box-sum, clamped borders) Trainium2 Bass kernel.

Input  x: [16, 3, 1024, 1024] f32, r=4 (hardcoded).
Output y: same shape; y[b,c,i,j] = sum of x[b,c,u,v] over the
(2r+1)x(2r+1) window centered at (i,j), clipped to the image bounds
(exactly what the reference's cumsum+diff computes).

Sharding: pure data parallel over 8 cores, 6 of the 48 images each.

Per-core pipeline (per image, 9 overlapping 128-row slabs):
  - The host splits x into bf16 hi/lo parts (x ~= hi + lo, residual
    <= 2^-18 relative) packed as [H, 2, W], so the H-direction matmul
    runs at bf16 speed (1 cycle/row vs 4 for fp32) with fp32-grade
    accuracy: band weights are exact 0/1 and PSUM accumulates in fp32.
  - H direction: banded 0/1 bf16 matmul on the TensorEngine. Slabs are
    chosen so each output-row block (124/120/60 rows) only needs input
    rows inside one 128-row slab -> 2 accumulating matmuls (hi+lo) per
    512-column PSUM bank, no cross-slab accumulation.
  - PSUM -> SBUF copies on the ScalarEngine, into a tile with 9
    leading and 4 trailing zero columns (zeroed once per pool slot:
    the first `bufs` allocations of a tag occupy distinct slots and
    the pads are never overwritten afterwards).
  - W direction: running 9-window sum on the VectorEngine via ONE
    merged tensor_tensor_scan over 1028 steps:
    state = (y[t] + state) - y[t-9] gives
    box_end[t] = sum_{k=max(0,t-8)}^{t} y[k] (the leading zero pad
    makes the left clamp automatic); for the last 4 steps data0 reads
    the trailing zero pad and data1 reads y[W-9..W-6], which walks
    the right clamp down from box_end[W-1]. Output col j (j < W-r) is
    box_end[j+r]; cols W-r.. come from the clamp walk.
  - Input DMAs issue from the Sync queue (pure prefetch stream),
    output DMAs from the GpSimd queue; both are HWDGE.
"""

import os
import numpy as np
import ml_dtypes

from concourse import bass, mybir, tile, bacc
from concourse.bass_utils import run_bass_kernel_spmd

F32 = mybir.dt.float32
BF16 = mybir.dt.bfloat16
H, W = 1024, 1024
N_CORES = 8
IPC = 6  # images per core: (16*3)/8
R = 4
D = 2 * R + 1  # 9

# slabs: (row0, nrows, out0, nouts, band_col)
_SLABS = (
    [(0, 128, 0, 124, 0)]
    + [(120 * i, 128, 120 * i + 4, 120, 124) for i in range(1, 8)]
    + [(960, 64, 964, 60, 244)]
)
_BAND_COLS = 304  # 124 + 120 + 60


def _band_matrix() -> np.ndarray:
    bands = np.zeros((128, _BAND_COLS), ml_dtypes.bfloat16)
    for row0, nrows, out0, nouts, bc in (_SLABS[0], _SLABS[1], _SLABS[8]):
        for j in range(nouts):
            h_out = out0 + j
            lo = max(0, h_out - R) - row0
            hi = min(H - 1, h_out + R) - row0
            bands[lo : hi + 1, bc + j] = 1.0
    return bands


_CACHE: dict = {}

# Set by the most recent kernel() call (for test harnesses).
LAST_RESULTS = None


def _build():
    nc = bacc.Bacc(
        "TRN2", target_bir_lowering=False, debug=False, enable_asserts=False
    )
    # hi/lo packed per row: x_hl[img, h, 0, :] = bf16 hi, [.., 1, :] = lo
    xhl_d = nc.dram_tensor("x_hl", [IPC, H, 2, W], BF16, kind="ExternalInput").ap()
    bands_d = nc.dram_tensor(
        "bands", [128, _BAND_COLS], BF16, kind="ExternalInput"
    ).ap()
    y_d = nc.dram_tensor("y", [IPC, H, W], F32, kind="ExternalOutput").ap()

    ADD = mybir.AluOpType.add
    SUB = mybir.AluOpType.subtract

    with tile.TileContext(nc) as tc:
        with (
            tc.tile_pool(name="const", bufs=1) as const_pool,
            tc.tile_pool(name="xin", bufs=12) as in_pool,
            tc.tile_pool(name="ps", bufs=8, space="PSUM") as ps_pool,
            tc.tile_pool(name="yrow", bufs=10) as y_pool,
            tc.tile_pool(name="box", bufs=12) as box_pool,
        ):
            bands_t = const_pool.tile([128, _BAND_COLS], BF16)
            nc.sync.dma_start(bands_t[:], bands_d[:])

            slab_idx = 0
            for img in range(IPC):
                for row0, nrows, out0, nouts, bc in _SLABS:
                    # [nrows, 2, 1024] -> [nrows part, 2048 free]: hi cols
                    # [0:1024), lo cols [1024:2048)
                    xhl = in_pool.tile([128, 2 * W], BF16, tag="xhl")
                    nc.sync.dma_start(
                        xhl[:nrows].rearrange("p (two w) -> p two w", two=2),
                        xhl_d[img, row0 : row0 + nrows, :, :],
                    )

                    # yt: [0:9) zeros, [9:1033) = H-filtered rows, [1033:1037)
                    # zeros (drives the right-border steps of the merged scan)
                    yt = y_pool.tile([128, W + D + R], F32, tag="yrow")
                    if slab_idx < 10:
                        # First `bufs` allocations occupy distinct pool slots;
                        # pads are never overwritten, so zero them once per
                        # physical buffer (full 128 partitions).
                        nc.vector.memset(yt[:, 0:D], 0.0)
                        nc.vector.memset(yt[:, D + W : D + W + R], 0.0)

                    band_ap = bands_t[:nrows, bc : bc + nouts]
                    for h in range(2):
                        ps = ps_pool.tile([128, 512], F32, tag="ps")
                        nc.tensor.matmul(
                            ps[:nouts],
                            lhsT=band_ap,
                            rhs=xhl[:nrows, h * 512 : (h + 1) * 512],
                            start=True,
                            stop=False,
                        )
                        nc.tensor.matmul(
                            ps[:nouts],
                            lhsT=band_ap,
                            rhs=xhl[:nrows, W + h * 512 : W + (h + 1) * 512],
                            start=False,
                            stop=True,
                        )
                        nc.scalar.copy(
                            yt[:nouts, D + h * 512 : D + (h + 1) * 512],
                            ps[:nouts],
                        )

                    # Merged scan: state = (y[t] + state) - y[t-9] over 1028
                    # steps. Steps 1024..1027 read data0 = 0 (tail pad) and
                    # data1 = y[W-9..W-6], which walks the right clamp down
                    # from box_end[W-1]. Output col j (j < W-r) = bx[j+r].
                    bx = box_pool.tile([128, W + R], F32, tag="box")
                    nc.vector.tensor_tensor_scan(
                        bx[:nouts, 0 : W + R],
                        yt[:nouts, D : D + W + R],
                        yt[:nouts, 0 : W + R],
                        0.0,
                        op0=ADD,
                        op1=SUB,
                    )
                    nc.gpsimd.dma_start(
                        y_d[img, out0 : out0 + nouts, :], bx[:nouts, R : R + W]
                    )
                    slab_idx += 1

    nc.compile()
    return nc


def kernel(x: np.ndarray, r) -> np.ndarray:
    global LAST_RESULTS
    x = np.asarray(x, dtype=np.float32)
    assert x.shape == (16, 3, H, W), x.shape
    assert int(r) == R, r

    nc = _CACHE.get("nc")
    if nc is None:
        nc = _CACHE["nc"] = _build()

    xr = x.reshape(N_CORES, IPC, H, W)
    x_hi = xr.astype(ml_dtypes.bfloat16)
    x_lo = (xr - x_hi.astype(np.float32)).astype(ml_dtypes.bfloat16)
    x_hl = np.stack([x_hi, x_lo], axis=3)  # [cores, IPC, H, 2, W]
    bands = _band_matrix()
    in_maps = [
        {"x_hl": np.ascontiguousarray(x_hl[c]), "bands": bands}
        for c in range(N_CORES)
    ]

    trace = bool(int(os.environ.get("BOX_TRACE", "0")))
    tmpdir = os.environ.get("BOX_TRACE_DIR") or None
    if tmpdir:
        os.makedirs(tmpdir, exist_ok=True)
    res = run_bass_kernel_spmd(
        nc, in_maps, list(range(N_CORES)), trace=trace, tmpdir=tmpdir
    )
    LAST_RESULTS = res
    y = np.stack([res.results[c]["y"] for c in range(N_CORES)])
    return y.reshape(16, 3, H, W)



# revision 2
# speedup vs baseline: 1.2404x; 1.2404x over previous
"""BoxFilter (9x9 box-sum, clamped borders) Trainium2 Bass kernel.

Input  x: [16, 3, 1024, 1024] f32, r=4 (hardcoded).
Output y: same shape; y[b,c,i,j] = sum of x[b,c,u,v] over the
(2r+1)x(2r+1) window centered at (i,j), clipped to the image bounds
(exactly what the reference's cumsum+diff computes).

Sharding: pure data parallel over 8 cores, 6 of the 48 images each.

The 2e-2 rel-err gate leaves huge headroom, so everything runs in
fp16 (input quantization + fp16 output give ~1e-3 rel err): input
DMA is 2 B/elem (no hi/lo split) and the output DMA is fp16 too,
upcast to f32 on the host.

Per-core pipeline (per image, 9 overlapping 128-row slabs). Each slab
is processed by one of three sub-pipelines, mixed to balance the
Vector, Tensor and Scalar engines (measured per-slab costs in ns:
scan 2274 V / extraction 1114 S / matmul 215 per 512-col stream T):

  A  (V-heavy): H-band matmul (2 MM) -> PSUM f32; ScalarE extracts to
     a zero-padded fp16 tile; one merged tensor_tensor_scan computes
     the 9-window running box along W (state=(y[t]+state)-y[t-9] over
     1028 steps; leading/trailing zero pads make both clamps
     automatic).
  B  (T-heavy): the full 2D box via 9 accumulating band-matmuls over
     column-shifted views of the zero-padded input slab (matmul cost
     is N cycles, K-independent, so the H window is free and each W
     shift costs one 512-col stream). ScalarE extraction is the
     final output. No Vector work.
  B2 (T+S): box9 = box3 o box3: 3 shifted band-matmuls -> t3 (PSUM,
     1030 cols across 3 banks), extract, then 3 shifted
     identity-matmuls of t3 -> final. 2.6us T + two extractions S,
     no V.

Input DMAs issue from the Sync queue, output DMAs from the GpSimd
queue; both are HWDGE.
"""

import os
import numpy as np

from concourse import bass, mybir, tile, bacc
from concourse.bass_utils import run_bass_kernel_spmd

F32 = mybir.dt.float32
FP16 = mybir.dt.float16
H, W = 1024, 1024
N_CORES = 8
IPC = 6  # images per core: (16*3)/8
R = 4
D = 2 * R + 1  # 9
XCOLS = W + 2 * R  # 1032: input slab with R zero cols each side

# slabs: (row0, nrows, out0, nouts, band_col)
_SLABS = (
    [(0, 128, 0, 124, 0)]
    + [(120 * i, 128, 120 * i + 4, 120, 124) for i in range(1, 8)]
    + [(960, 64, 964, 60, 244)]
)
_BAND_COLS = 304  # 124 + 120 + 60

# slab type per (img, slab): A=scan, B=9-shift matmul, B2=3+3 two-level
_PAT1 = ["A", "B2", "A", "A", "B", "A", "B2", "A", "A"]
_PAT2 = ["A", "B2", "A", "B2", "B", "A", "B2", "A", "A"]
_PATTERNS = [_PAT1, _PAT2, _PAT1, _PAT2, _PAT1, _PAT1]


def _band_matrix() -> np.ndarray:
    bands = np.zeros((128, _BAND_COLS), np.float16)
    for row0, nrows, out0, nouts, bc in (_SLABS[0], _SLABS[1], _SLABS[8]):
        for j in range(nouts):
            h_out = out0 + j
            lo = max(0, h_out - R) - row0
            hi = min(H - 1, h_out + R) - row0
            bands[lo : hi + 1, bc + j] = 1.0
    return bands


_CACHE: dict = {}

# Set by the most recent kernel() call (for test harnesses).
LAST_RESULTS = None


def _build():
    nc = bacc.Bacc(
        "TRN2", target_bir_lowering=False, debug=False, enable_asserts=False
    )
    x_d = nc.dram_tensor("x", [IPC, H, W], FP16, kind="ExternalInput").ap()
    bands_d = nc.dram_tensor(
        "bands", [128, _BAND_COLS], FP16, kind="ExternalInput"
    ).ap()
    ident_d = nc.dram_tensor("ident", [128, 128], FP16, kind="ExternalInput").ap()
    y_d = nc.dram_tensor("y", [IPC, H, W], FP16, kind="ExternalOutput").ap()

    ADD = mybir.AluOpType.add
    SUB = mybir.AluOpType.subtract

    XPAD_BUFS = 8
    YT_BUFS = 5
    BX_BUFS = 5
    OUT_BUFS = 5
    T3_BUFS = 3

    with tile.TileContext(nc) as tc:
        with (
            tc.tile_pool(name="const", bufs=1) as const_pool,
            tc.tile_pool(name="xin", bufs=XPAD_BUFS) as in_pool,
            tc.tile_pool(name="ps2", bufs=2, space="PSUM") as ps2_pool,
            tc.tile_pool(name="ps3", bufs=1, space="PSUM") as ps3_pool,
            tc.tile_pool(name="yrow", bufs=YT_BUFS) as y_pool,
            tc.tile_pool(name="box", bufs=BX_BUFS) as box_pool,
            tc.tile_pool(name="t3", bufs=T3_BUFS) as t3_pool,
            tc.tile_pool(name="outb", bufs=OUT_BUFS) as out_pool,
        ):
            bands_t = const_pool.tile([128, _BAND_COLS], FP16)
            nc.sync.dma_start(bands_t[:], bands_d[:])
            ident_t = const_pool.tile([128, 128], FP16)
            nc.sync.dma_start(ident_t[:], ident_d[:])

            slab_idx = 0
            for img in range(IPC):
                pat = _PATTERNS[img]
                for si, (row0, nrows, out0, nouts, bc) in enumerate(_SLABS):
                    typ = pat[si]
                    band_ap = bands_t[:nrows, bc : bc + nouts]

                    # input slab with R zero cols each side (pads zeroed
                    # once per physical pool slot)
                    xpad = in_pool.tile([128, XCOLS], FP16, tag="xpad")
                    if slab_idx < XPAD_BUFS:
                        nc.vector.memset(xpad[:, 0:R], 0.0)
                        nc.vector.memset(xpad[:, R + W : XCOLS], 0.0)
                    nc.sync.dma_start(
                        xpad[:nrows, R : R + W],
                        x_d[img, row0 : row0 + nrows, :],
                    )

                    if typ == "A":
                        ps = ps2_pool.tile([128, 1024], F32, tag="ps2")
                        for b in range(2):
                            nc.tensor.matmul(
                                ps[:nouts, b * 512 : (b + 1) * 512],
                                lhsT=band_ap,
                                rhs=xpad[:nrows, R + b * 512 : R + (b + 1) * 512],
                                start=True,
                                stop=True,
                            )
                        # yt: [0:9) zeros, [9:1033) = H-filtered rows,
                        # [1033:1037) zeros (right-border steps of the scan)
                        yt = y_pool.tile([128, W + D + R], FP16, tag="yrow")
                        if slab_idx < YT_BUFS * 2:
                            nc.vector.memset(yt[:, 0:D], 0.0)
                            nc.vector.memset(yt[:, D + W : D + W + R], 0.0)
                        nc.scalar.copy(yt[:nouts, D : D + W], ps[:nouts, :])
                        # merged scan: state = (y[t] + state) - y[t-9]; for
                        # the last 4 steps data0 reads the trailing zeros and
                        # data1 walks the right clamp down from box_end[W-1].
                        bx = box_pool.tile([128, W + R], FP16, tag="box")
                        nc.vector.tensor_tensor_scan(
                            bx[:nouts, 0 : W + R],
                            yt[:nouts, D : D + W + R],
                            yt[:nouts, 0 : W + R],
                            0.0,
                            op0=ADD,
                            op1=SUB,
                        )
                        nc.gpsimd.dma_start(
                            y_d[img, out0 : out0 + nouts, :],
                            bx[:nouts, R : R + W],
                        )

                    elif typ == "B":
                        ps = ps2_pool.tile([128, 1024], F32, tag="ps2")
                        for b in range(2):
                            for s in range(D):
                                nc.tensor.matmul(
                                    ps[:nouts, b * 512 : (b + 1) * 512],
                                    lhsT=band_ap,
                                    rhs=xpad[:nrows, s + b * 512 : s + b * 512 + 512],
                                    start=(s == 0),
                                    stop=(s == D - 1),
                                )
                        ob = out_pool.tile([128, W], FP16, tag="outb")
                        nc.scalar.copy(ob[:nouts, :], ps[:nouts, :])
                        nc.gpsimd.dma_start(
                            y_d[img, out0 : out0 + nouts, :], ob[:nouts, :]
                        )

                    else:  # B2: box9 = box3 of box3
                        # level 1: t3[m] = T[m-1]+T[m]+T[m+1] for
                        # m = -3..1026 (psum col c = m+3, 1030 cols over 3
                        # banks); clamping comes from the xpad zeros.
                        ps3 = ps3_pool.tile([128, 1536], F32, tag="ps3")
                        for c0, n in ((0, 512), (512, 512), (1024, 6)):
                            for s in range(3):
                                nc.tensor.matmul(
                                    ps3[:nouts, c0 : c0 + n],
                                    lhsT=band_ap,
                                    rhs=xpad[:nrows, s + c0 : s + c0 + n],
                                    start=(s == 0),
                                    stop=(s == 2),
                                )
                        t3b = t3_pool.tile([128, 1030], FP16, tag="t3b")
                        nc.scalar.copy(t3b[:nouts, :], ps3[:nouts, 0:1030])
                        # level 2: out[j] = t3[j-3] + t3[j] + t3[j+3]
                        #        = sum_{s in {0,3,6}} t3b[:, j + s]
                        ps = ps2_pool.tile([128, 1024], F32, tag="ps2")
                        for b in range(2):
                            for s in (0, 3, 6):
                                nc.tensor.matmul(
                                    ps[:nouts, b * 512 : (b + 1) * 512],
                                    lhsT=ident_t[:nouts, :nouts],
                                    rhs=t3b[:nouts, s + b * 512 : s + b * 512 + 512],
                                    start=(s == 0),
                                    stop=(s == 6),
                                )
                        ob = out_pool.tile([128, W], FP16, tag="outb")
                        nc.scalar.copy(ob[:nouts, :], ps[:nouts, :])
                        nc.gpsimd.dma_start(
                            y_d[img, out0 : out0 + nouts, :], ob[:nouts, :]
                        )

                    slab_idx += 1

    nc.compile()
    return nc


def kernel(x: np.ndarray, r) -> np.ndarray:
    global LAST_RESULTS
    x = np.asarray(x, dtype=np.float32)
    assert x.shape == (16, 3, H, W), x.shape
    assert int(r) == R, r

    nc = _CACHE.get("nc")
    if nc is None:
        nc = _CACHE["nc"] = _build()

    xr = x.reshape(N_CORES, IPC, H, W).astype(np.float16)
    bands = _band_matrix()
    ident = np.eye(128, dtype=np.float16)
    in_maps = [
        {"x": np.ascontiguousarray(xr[c]), "bands": bands, "ident": ident}
        for c in range(N_CORES)
    ]

    trace = bool(int(os.environ.get("BOX_TRACE", "0")))
    tmpdir = os.environ.get("BOX_TRACE_DIR") or None
    if tmpdir:
        os.makedirs(tmpdir, exist_ok=True)
    res = run_bass_kernel_spmd(
        nc, in_maps, list(range(N_CORES)), trace=trace, tmpdir=tmpdir
    )
    LAST_RESULTS = res
    y = np.stack([res.results[c]["y"] for c in range(N_CORES)])
    return y.reshape(16, 3, H, W).astype(np.float32)
